# revision 1
# baseline (speedup 1.0000x reference)
"""DiffAttention kernel for 8 TRN2 NeuronCores (Bass/Tile).

Reference computation (see problem): x [1,128,32,32,32] is stride-2
subsampled to xs [128, N=4096 tokens]; qkv = w_qkv @ xs per head
(4 heads, head_dim 32, split into two halves of 16 for the two
softmaxes); diff_attn = softmax(q1k1) - 0.1*softmax(q2k2); out = diff
attn @ v, reshaped back to [1,128,16,16,16].

Sharding: tensor-parallel over (head, query-half) = 8 shards, one per
core. Each core computes its head's full K/V over all 4096 tokens and
attention for its 2048 queries.

Per-core dataflow (all on-chip, flash-style, no NxN HBM traffic):
  - scores are computed TRANSPOSED, sT[m,n] = k^T q, so the softmax
    denominator can be folded into the AV matmul via a ones-column
    appended to v^T; k1/q1 live on partition strip 32:48 and k2/q2 on
    64:80 so the two score matmuls row-pair on the array.
  - exp is SPLIT across two engines so neither paces the loop: ACT
    exps the s1 half (exact, feeds the dominant softmax), DVE exps the
    s2 half with the Schraudolph bit trick -- one tensor_scalar
    computing int16(round(128*log2e*scale*s + 16248.59)) whose bits
    ARE bf16(exp(s*scale)) to within +-3%; that error enters the
    output attenuated by lambda=0.1.
  - the m-loop is processed in GROUPS of 8: [8 score pairs + exps]
    then [the previous group's 8 av pairs].  Long uniform runs let the
    PE's fast streaming mode engage (hw-measured: alternating
    score/av pairs every iteration streams at ~427 ns/pair, batched
    runs reach ~216 ns/pair); all projection matmuls and the finalize
    transposes are confined to group boundaries for the same reason.
  - AV: out^T[d,n] accumulated over m-chunks in PSUM; AV1 at psum
    partitions 0:33, AV2 at 64:97 (col-tiled pair).
  - finalize: PE-transpose av -> [n,33], per-partition reciprocal of
    the sum column, combine out = av1/s1 - 0.1*av2/s2 on DVE.
"""

import math

import numpy as np
import ml_dtypes

import concourse.bass as bass
import concourse.mybir as mybir
import concourse.tile as tile
from concourse import bacc
from concourse.bass import ts, ds
from concourse.bass_utils import run_bass_kernel_spmd

BF16 = mybir.dt.bfloat16
F32 = mybir.dt.float32
I16 = mybir.dt.int16
NP_BF16 = ml_dtypes.bfloat16

C = 128          # channels
HEADS = 4
HD = 32          # head_dim
DH = 16          # d_half
LAMBDA = 0.1
SCALE = HD ** -0.5
R = 2
N_CORES = 8
N = 4096         # tokens after subsample
NQ = N // 2      # queries per core

# Schraudolph constants: int16 bits of bf16(2^y) ~= 128*y + 128*(127-c),
# c = 0.0579297 balances the max relative error at ~+-2.98%; the DVE
# f32->int16 store rounds to nearest (hw-verified).
SCH_A = 128.0 * math.log2(math.e) * SCALE
SCH_B = 128.0 * (127.0 - 0.0579297)

# per-iteration exp column split (of the 1024-col sj tile): ACT does
# [0:CA], DVE does [CA:1024].  511 so av1's rhs [0:512] overlaps the
# DVE range and carries both exp semaphores.  During j-block 0 the DVE
# also absorbs the k projection copies, so ACT takes more columns.
CA = 511
CA_J0 = 608

GB = 8           # m-chunks per group
MC = N // 128    # 32 m-chunks
NG = MC // GB    # 4 groups per j-block
NJ = NQ // 512   # 4 j-blocks per core
NBS = 1024       # queries per av accumulator block (2 j-blocks)

# weight tensor column layout (w input, [128, 96]):
WV = slice(0, 32)     # w_v^T   (rhs of vT matmuls)
WK1 = slice(32, 48)   # w_k1^T
WK2 = slice(48, 64)   # w_k2^T
WQ1 = slice(64, 80)   # w_q1^T
WQ2 = slice(80, 96)   # w_q2^T


def build_nc(NT=N, NQL=NQ):
    """Build the SPMD Bass program for one core = (head, query-half).

    Per-core inputs:
      xs    [128, NT]   bf16  all tokens, channel-major (for K and V)
      xq    [128, NQL]  bf16  this core's query tokens
      w     [128, 96]   bf16  columns per WV/WK1/WK2/WQ1/WQ2 slices
      ident [128, 33]   f32   identity blocks at partitions 0:33, 64:97
    Output:
      out   [NQL, 32]   f32   attention output (n, d) for the queries
    """
    Exp = mybir.ActivationFunctionType.Exp

    nc = bacc.Bacc()
    xs_d = nc.declare_dram_parameter("xs", [C, NT], BF16, isOutput=False)
    xq_d = nc.declare_dram_parameter("xq", [C, NQL], BF16, isOutput=False)
    w_d = nc.declare_dram_parameter("w", [C, 96], BF16, isOutput=False)
    id_d = nc.declare_dram_parameter("ident", [C, 33], F32, isOutput=False)
    out_d = nc.declare_dram_parameter("out", [NQL, HD], F32, isOutput=True)

    with tile.TileContext(nc) as tc:
        with (
            tc.tile_pool(name="consts", bufs=1) as consts,
            tc.tile_pool(name="mains", bufs=1) as mains,
        ):
            w_sb = consts.tile([C, 96], BF16)
            nc.sync.dma_start(out=w_sb[:, :], in_=w_d[:, :])
            id_sb = consts.tile([C, 33], F32)
            nc.sync.dma_start(out=id_sb[:, :], in_=id_d[:, :])

            def chunked_dma(eng, dst, src, total):
                sizes, rem = [], total
                for sz in (512, 512, 1024):
                    if rem >= sz:
                        sizes.append(sz)
                        rem -= sz
                while rem > 0:
                    sz = 2048 if rem >= 2048 else 512
                    sizes.append(sz)
                    rem -= sz
                off = 0
                for sz in sizes:
                    eng.dma_start(out=dst[:, ds(off, sz)],
                                  in_=src[:, ds(off, sz)])
                    off += sz

            xs_sb = mains.tile([C, NT], BF16)
            chunked_dma(nc.gpsimd, xs_sb, xs_d, NT)
            xq_sb = mains.tile([C, NQL], BF16)
            chunked_dma(nc.scalar, xq_sb, xq_d, NQL)

            kk_sb = mains.tile([C, NT], BF16)    # parts 32:48 k1, 64:80 k2
            qq_sb = mains.tile([C, NQL], BF16)   # parts 32:48 q1, 64:80 q2
            vTa_sb = mains.tile([C, MC * 33], BF16)  # per chunk: v^T | ones
            av_sb = mains.tile([C, 2 * NBS], F32)  # parts 0:33 AV1|s1, 64:97 AV2|s2
            out_sb = mains.tile([C, (NQL // 128) * HD], F32)

            nc.vector.memset(vTa_sb[:, :], 1.0)

            with (
                tc.tile_pool(name="sj_ps", bufs=3, space="PSUM") as spool,
                tc.tile_pool(name="av_ps", bufs=1, space="PSUM") as avpool,
                tc.tile_pool(name="e_sb", bufs=18) as epool,
                tc.tile_pool(name="fin_sb", bufs=2) as fsb,
            ):
                def project_q(t):
                    # q chunk t = queries for j-block t
                    ps_q = spool.tile([C, 1024], F32, tag="sj", name="ps_q")
                    nc.tensor.matmul(ps_q[32:48, 0:512], lhsT=w_sb[:, WQ1],
                                     rhs=xq_sb[:, ts(t, 512)],
                                     start=True, stop=True)
                    nc.tensor.matmul(ps_q[64:80, 0:512], lhsT=w_sb[:, WQ2],
                                     rhs=xq_sb[:, ts(t, 512)],
                                     start=True, stop=True)
                    nc.vector.tensor_copy(qq_sb[32:48, ts(t, 512)],
                                          ps_q[32:48, 0:512])
                    nc.vector.tensor_copy(qq_sb[64:80, ts(t, 512)],
                                          ps_q[64:80, 0:512])

                def project_kv(t):
                    # k chunk t = keys for m-chunks 4t..4t+3
                    ps_kv = spool.tile([C, 1024], F32, tag="sj", name="ps_kv")
                    nc.tensor.matmul(ps_kv[32:48, 0:512], lhsT=w_sb[:, WK1],
                                     rhs=xs_sb[:, ts(t, 512)],
                                     start=True, stop=True)
                    nc.tensor.matmul(ps_kv[64:80, 0:512], lhsT=w_sb[:, WK2],
                                     rhs=xs_sb[:, ts(t, 512)],
                                     start=True, stop=True)
                    nc.vector.tensor_copy(kk_sb[32:48, ts(t, 512)],
                                          ps_kv[32:48, 0:512])
                    nc.vector.tensor_copy(kk_sb[64:80, ts(t, 512)],
                                          ps_kv[64:80, 0:512])

                def project_vt(m):
                    ps_vt = spool.tile([C, 1024], F32, tag="sj", name="ps_vt")
                    nc.tensor.matmul(ps_vt[:, 0:HD], lhsT=xs_sb[:, ts(m, 128)],
                                     rhs=w_sb[:, WV], start=True, stop=True)
                    nc.scalar.copy(vTa_sb[:, ds(m * 33, HD)], ps_vt[:, 0:HD])

                def finalize_nb(nb):
                    # transpose av -> [n, 33], reciprocal of the sum
                    # column, combine out = av1/s1 - 0.1*av2/s2 on DVE
                    CQ = NBS // 128  # 8 query chunks of 128
                    psT1 = spool.tile([C, 1024], F32, tag="sj", name="psT1")
                    psT2 = spool.tile([C, 1024], F32, tag="sj", name="psT2")
                    for cq in range(CQ):
                        gq = nb * CQ + cq
                        nc.tensor.transpose(psT1[:, ds(cq * 64, 33)],
                                            av_sb[0:33, ts(gq, 128)],
                                            id_sb[0:33, :])
                        nc.tensor.transpose(psT2[:, ds(cq * 64, 33)],
                                            av_sb[64:97, ts(gq, 128)],
                                            id_sb[64:97, :])
                    r1_sb = fsb.tile([C, CQ], F32, tag="r1")
                    r2_sb = fsb.tile([C, CQ], F32, tag="r2")
                    sum1 = psT1[:, 0:CQ * 64].rearrange(
                        "p (c x) -> p c x", x=64)[:, :, 32:33]
                    sum2 = psT2[:, 0:CQ * 64].rearrange(
                        "p (c x) -> p c x", x=64)[:, :, 32:33]
                    nc.vector.reciprocal(r1_sb[:, :, None], sum1)
                    nc.vector.reciprocal(r2_sb[:, :, None], sum2)
                    nc.vector.tensor_scalar_mul(r2_sb[:, :], r2_sb[:, :],
                                                -LAMBDA)
                    o1_sb = fsb.tile([C, CQ * HD], F32, tag="o1")
                    o2_sb = fsb.tile([C, CQ * HD], F32, tag="o2")
                    av1t = psT1[:, 0:CQ * 64].rearrange(
                        "p (c x) -> p c x", x=64)[:, :, 0:32]
                    av2t = psT2[:, 0:CQ * 64].rearrange(
                        "p (c x) -> p c x", x=64)[:, :, 0:32]
                    o1_v = o1_sb[:, :].rearrange("p (c d) -> p c d", d=HD)
                    o2_v = o2_sb[:, :].rearrange("p (c d) -> p c d", d=HD)
                    nc.vector.tensor_tensor(
                        o1_v, av1t,
                        r1_sb[:, :, None].to_broadcast((C, CQ, HD)),
                        mybir.AluOpType.mult)
                    nc.vector.tensor_tensor(
                        o2_v, av2t,
                        r2_sb[:, :, None].to_broadcast((C, CQ, HD)),
                        mybir.AluOpType.mult)
                    nc.vector.tensor_tensor(
                        out_sb[:, ds(nb * CQ * HD, CQ * HD)],
                        o1_sb[:, :], o2_sb[:, :], mybir.AluOpType.add)
                    out_view = out_d[:, :].rearrange("(c p) d -> p c d", p=C)
                    nc.sync.dma_start(
                        out=out_view[:, nb * CQ:(nb + 1) * CQ, :],
                        in_=out_sb[:, ds(nb * CQ * HD, CQ * HD)]
                            .rearrange("p (c d) -> p c d", d=HD),
                    )

                # minimal chain to the first scores: k chunks 0,1 and
                # the first j-block's queries
                project_kv(0)
                project_q(0)
                project_kv(1)

                pending_av = None      # av batch closure of previous group
                pending_fin = None     # finalize closure of previous n-block

                for j in range(NJ):
                    nb = j // 2
                    for g in range(NG):
                        # ---- boundary work (all PE disturbances live
                        # here): projections one group ahead + finalize
                        if j == 0:
                            if g >= 1:
                                project_kv(2 * g)
                                project_kv(2 * g + 1)
                            # v^T for this group's m-range (consumed by
                            # this group's av batch)
                            for m in range(g * GB, g * GB + GB):
                                project_vt(m)
                        if j == 0 and g == 2:
                            project_q(1)
                        if j == 1 and g == 2:
                            project_q(2)
                        if j == 2 and g == 0 and pending_fin is not None:
                            pending_fin()
                            pending_fin = None
                        if j == 2 and g == 2:
                            project_q(3)

                        # ---- s batch: 8 score pairs + split exps
                        ca = CA_J0 if j == 0 else CA
                        e_tiles = []
                        for m in range(g * GB, g * GB + GB):
                            nsl = ds(j * 512, 512)
                            sj_ps = spool.tile([C, 1024], F32, tag="sj")
                            nc.tensor.matmul(sj_ps[:, 0:512],
                                             lhsT=kk_sb[32:48, ts(m, 128)],
                                             rhs=qq_sb[32:48, nsl],
                                             start=True, stop=True)
                            nc.tensor.matmul(sj_ps[:, 512:1024],
                                             lhsT=kk_sb[64:80, ts(m, 128)],
                                             rhs=qq_sb[64:80, nsl],
                                             start=True, stop=True)
                            e_sb = epool.tile([C, 1024], BF16, tag="e")
                            nc.scalar.activation(e_sb[:, 0:ca],
                                                 sj_ps[:, 0:ca], Exp,
                                                 scale=SCALE)
                            nc.vector.tensor_scalar(
                                e_sb[:, ca:1024].bitcast(I16),
                                sj_ps[:, ca:1024], SCH_A, SCH_B,
                                mybir.AluOpType.mult, mybir.AluOpType.add)
                            e_tiles.append((m, e_sb))

                        # ---- release gate + previous group's av batch.
                        # The gate multiplies the pending batch's vTa
                        # ones-column cells by 1.0 (value-preserving);
                        # sitting after this batch's exps in DVE program
                        # order, it makes all 16 avs become ready
                        # ATOMICALLY after this s batch finishes: they
                        # run as one uniform burst instead of being
                        # picked one-by-one into the s batch's exp-paced
                        # stalls by the scheduler's ready-heap.
                        if pending_av is not None:
                            fn, m_lo = pending_av
                            ones_view = vTa_sb[0:1, ds(m_lo * 33, GB * 33)] \
                                .rearrange("p (c x) -> p c x", x=33)[:, :, 32:33]
                            nc.vector.tensor_scalar(
                                ones_view, ones_view, 1.0, None,
                                mybir.AluOpType.mult)
                            fn()
                            pending_av = None

                        def av_batch(e_tiles=e_tiles, j=j, av_ps=None):
                            # emission order [m_lo, m_hi, m_lo+1, ...]:
                            # the second pair carries the batch's highest
                            # exp semaphores, so the remaining pairs'
                            # waits are subsumed and the burst streams
                            # without per-pair wait gaps.  stop lands on
                            # the last EMITTED matmul (sim bookkeeping);
                            # psum accumulation order is free.
                            order = ([e_tiles[0], e_tiles[-1]]
                                     + e_tiles[1:-1])
                            n_em = len(order)
                            for i, (m, e_sb) in enumerate(order):
                                first = (m % MC == 0)
                                last = (i == n_em - 1 and
                                        order[i][0] % MC >= MC - GB and
                                        max(mm for mm, _ in order) == MC - 1)
                                nc.tensor.matmul(
                                    av_ps[0:33, ts(j % 2, 512)],
                                    lhsT=vTa_sb[:, ds(m * 33, 33)],
                                    rhs=e_sb[:, 0:512],
                                    start=first, stop=last,
                                    skip_group_check=True)
                                nc.tensor.matmul(
                                    av_ps[64:97, ts(j % 2, 512)],
                                    lhsT=vTa_sb[:, ds(m * 33, 33)],
                                    rhs=e_sb[:, 512:1024],
                                    start=first, stop=last,
                                    skip_group_check=True)
                        if j % 2 == 0 and g == 0:
                            av_tile = avpool.tile([C, NBS], F32, tag="av")
                        pending_av = ((lambda f=av_batch, t=av_tile:
                                       f(av_ps=t)), g * GB)

                    # ---- end of j-block
                    if j % 2 == 1:
                        # flush the block's last av batch, then drain
                        pending_av[0]()
                        pending_av = None
                        nc.vector.tensor_copy(
                            av_sb[0:33, ds(nb * NBS, NBS)], av_tile[0:33, :])
                        nc.scalar.copy(
                            av_sb[64:97, ds(nb * NBS, NBS)], av_tile[64:97, :])
                        if j == NJ - 1:
                            finalize_nb(nb)
                        else:
                            pending_fin = (lambda nb=nb: finalize_nb(nb))

    nc.compile()
    return nc


def make_identity_input():
    ident = np.zeros((C, 33), np.float32)
    ident[0:33, :] = np.eye(33, dtype=np.float32)
    ident[64:97, :] = np.eye(33, dtype=np.float32)
    return ident


def make_in_maps(x, w_qkv):
    """Host-side sharding: subsample, pack per-core inputs."""
    xs = np.ascontiguousarray(x[0][:, ::R, ::R, ::R]).reshape(C, N)
    xs_b = xs.astype(NP_BF16)
    ident = make_identity_input()
    in_maps = []
    for core in range(N_CORES):
        h, half = divmod(core, 2)
        wq = w_qkv[h * 96: h * 96 + 32]       # [32, 128]
        wk = w_qkv[h * 96 + 32: h * 96 + 64]
        wv = w_qkv[h * 96 + 64: h * 96 + 96]
        w = np.empty((C, 96), np.float32)
        w[:, WV] = wv.T
        w[:, WK1] = wk[0:DH].T
        w[:, WK2] = wk[DH:HD].T
        w[:, WQ1] = wq[0:DH].T
        w[:, WQ2] = wq[DH:HD].T
        in_maps.append({
            "xs": xs_b,
            "xq": np.ascontiguousarray(xs_b[:, half * NQ:(half + 1) * NQ]),
            "w": w.astype(NP_BF16),
            "ident": ident,
        })
    return in_maps


_NC_CACHE = {}


def get_nc():
    if "nc" not in _NC_CACHE:
        _NC_CACHE["nc"] = build_nc()
    return _NC_CACHE["nc"]


LAST_RESULTS = None  # BassKernelResults of the most recent kernel() call


def kernel(x, w_qkv, trace=False, **trace_kwargs):
    global LAST_RESULTS
    x = np.asarray(x)
    w_qkv = np.asarray(w_qkv)
    in_maps = make_in_maps(x, w_qkv)
    nc = get_nc()
    res = run_bass_kernel_spmd(nc, in_maps, list(range(N_CORES)),
                               trace=trace, **trace_kwargs)
    LAST_RESULTS = res
    out_hnd = np.empty((HEADS, N, HD), np.float32)
    for core in range(N_CORES):
        h, half = divmod(core, 2)
        out_hnd[h, half * NQ:(half + 1) * NQ, :] = res.results[core]["out"]
    return out_hnd.reshape(1, C, 16, 16, 16)



# revision 6
# speedup vs baseline: 1.2510x; 1.2510x over previous
"""DiffAttention kernel for 8 TRN2 NeuronCores (Bass/Tile).

Reference: x [1,128,32,32,32] stride-2 subsampled to xs [128, N=4096
tokens]; per head (4 heads, head_dim 32 split 16+16): diff_attn =
softmax(q1k1*scale) - 0.1*softmax(q2k2*scale); out = diff_attn @ v,
reshaped to [1,128,16,16,16].

Sharding: tensor-parallel over (head, query-half) = 8 cores. Each core
holds all 4096 tokens and computes attention for its 2048 queries.

Per-core dataflow (v2 — PE-paced, fp8 AV):
  - score weights folded on host: W1 = c8*wk1^T wq1, W2 = c8*wk2^T wq2
    (c8 = 8*log2e*scale pre-scales scores so the fp8 Schraudolph is a
    plain add).  s_h = xs_chunk^T @ (W_h @ xs_qslice): no k projection,
    one shared lhsT (the xs chunk) for both halves' score matmuls.
  - exp is whole-chunk alternated between ACT (exact exp -> fp8e4m3,
    scale ln2/8) and DVE (Schraudolph: uint8 bits = c8*s + 8*(7-c),
    round-to-nearest, min-clamp 119 to avoid inf/NaN), weighted so
    both engines run just under the PE's pace.
  - AV in fp8: av1 = vT^T @ e1 per chunk (plain fp8 matmul), av2 via
    DoubleRow over chunk PAIRS (lhsT [128,2,128] zero-padded cols
    33:128, rhs = two adjacent e-slots viewed [128,2,512]) at 0.5
    cycles/col.  A ones-column in vT accumulates the softmax sums.
  - psum: 3 score tiles [128,1024] (s1|s2 per chunk) + av1 bank +
    av2 bank = 8 banks.  av banks are single-buffered per j-block
    (start=True resets); finalize transposes reuse them between
    j-blocks.
  - keeping the PE gaplessly busy (scores+av ~1385ns per chunk pair >
    exp ~1240ns) holds the fast p-state (0.42ns/col after 3us).
"""

import math

import numpy as np
import ml_dtypes

import concourse.bass as bass
import concourse.mybir as mybir
import concourse.tile as tile
from concourse import bacc
from concourse.bass import ts, ds
from concourse.bass_utils import run_bass_kernel_spmd

BF16 = mybir.dt.bfloat16
I16 = mybir.dt.int16
F32 = mybir.dt.float32
FP8 = mybir.dt.float8e4
U8 = mybir.dt.uint8
NP_BF16 = ml_dtypes.bfloat16
NP_FP8 = ml_dtypes.float8_e4m3

C = 128          # channels
HEADS = 4
HD = 32          # head_dim
DH = 16          # d_half
LAMBDA = 0.1
SCALE = HD ** -0.5
R = 2
N_CORES = 8
N = 4096         # tokens after subsample
NQ = N // 2      # queries per core

MC = N // 128    # 32 key chunks of 128 tokens
ND = MC // 2     # 16 chunk pairs (double-chunks)
NJ = NQ // 512   # 4 j-blocks of 512 queries
NBS = 1024       # queries per finalize block (2 j-blocks)

# fp8e4m3 (ml_dtypes.float8_e4m3, IEEE-ish: bias 7, max 240, inf at
# bits 0x78): bits(2^y) ~= 8*(y + 7 - c), c balancing the Schraudolph
# sawtooth.  Scores are pre-scaled by C8 in the W weights so ACT
# recovers exp(scale*s) with scale ln2/8 and DVE just adds SCH8_B.
C8 = 8.0 * math.log2(math.e) * SCALE
ACT_SCALE = math.log(2.0) / 8.0
# global exponent shift (softmax-invariant): keeps exp values inside
# fp8e4m3 range (max 240; scores reach exp(6.5) otherwise)
SHIFT = 2.0
SCH8_B = 8.0 * (7.0 - 0.0579297) - 8.0 * math.log2(math.e) * SHIFT
CLAMP8 = 119.49
# bf16 Schraudolph for e1 chunks: bits = 16*ps + SCH16_B (psum is c8*s)
SCH16_A = 16.0
SCH16_B = 128.0 * (127.0 - 0.0579297) - 128.0 * math.log2(math.e) * SHIFT

# exp engine split: fraction of chunks on ACT (rest on DVE-Schraudolph)
ACT_SHARE = 0.54

ESLOTS = 8       # e8 ring slots (chunks)


def build_nc():
    """SPMD Bass program for one core = (head, query-half).

    Inputs:
      xs    [128, 4096] bf16  all tokens, channel-major
      w     [128, 288]  bf16  cols 0:128 W1^T, 128:256 W2^T, 256:288 w_v^T
      ident [128, 33]   f32   identity blocks at partitions 0:33, 64:97
    Output:
      out   [2048, 32]  f32   attention output (n, d) for this core's
                              queries
    """
    Exp = mybir.ActivationFunctionType.Exp
    DR = mybir.MatmulPerfMode.DoubleRow

    nc = bacc.Bacc()
    xs_d = nc.declare_dram_parameter("xs", [C, N], BF16, isOutput=False)
    xq_d = nc.declare_dram_parameter("xq", [C, NQ], BF16, isOutput=False)
    w_d = nc.declare_dram_parameter("w", [C, 288], BF16, isOutput=False)
    id_d = nc.declare_dram_parameter("ident", [C, 33], F32, isOutput=False)
    out_d = nc.declare_dram_parameter("out", [NQ, HD], F32, isOutput=True)

    W1 = slice(0, 128)
    W2 = slice(128, 256)
    WV = slice(256, 288)

    with tile.TileContext(nc) as tc:
        with tc.tile_pool(name="mains", bufs=1) as mains:
            w_sb = mains.tile([C, 288], BF16)
            nc.sync.dma_start(out=w_sb[:, :], in_=w_d[:, :])
            id_sb = mains.tile([C, 33], F32)
            nc.sync.dma_start(out=id_sb[:, :], in_=id_d[:, :])

            xs_sb = mains.tile([C, N], BF16)
            # chunked DMA so projections can start early
            for i, (off, sz) in enumerate(
                    [(0, 512), (512, 512), (1024, 1024), (2048, 2048)]):
                eng = nc.gpsimd if i % 2 == 0 else nc.scalar
                eng.dma_start(out=xs_sb[:, ds(off, sz)],
                              in_=xs_d[:, ds(off, sz)])
            xq_sb = mains.tile([C, NQ], BF16)
            for off in (0, 512, 1024, 1536):
                nc.gpsimd.dma_start(out=xq_sb[:, ds(off, 512)],
                                    in_=xq_d[:, ds(off, 512)])

            # static tensors
            qq_sb = mains.tile([C, NJ * 2 * 512], BF16)   # t1|t2 per j
            vta1_sb = mains.tile([C, MC * 33], BF16)      # av1 weights v|1
            vta8_sb = mains.tile([C, ND * 2 * 128], FP8)  # av2 DR weights
            e1_sb = mains.tile([C, ESLOTS * 512], BF16)   # e1 ring
            e2_sb = mains.tile([C, ESLOTS * 512], FP8)    # e2 ring
            avs_sb = mains.tile([C, NJ * 512], F32)       # av1 p0:33, av2 p64:97
            out_sb = mains.tile([C, (NQ // 128) * HD], F32)

            vta8_v = vta8_sb[:, :].rearrange("p (d s m) -> p d s m",
                                             d=ND, s=2)
            qq_v = qq_sb[:, :].rearrange("p (j s n) -> p j s n",
                                         j=NJ, s=2)

            # activation bias AP (-SHIFT) for the exact-exp path
            bias_sb = mains.tile([C, 1], F32)
            nc.vector.memset(bias_sb[:, :], -SHIFT)
            # ones columns + zero DR pad cols, written once
            nc.vector.memset(vta1_sb[:, :], 1.0)
            nc.vector.memset(vta8_sb[:, :], 0.0)
            nc.vector.memset(vta8_v[:, :, :, 32:33], 1.0)

            with (
                tc.tile_pool(name="sc_ps", bufs=3, space="PSUM") as spool,
                tc.tile_pool(name="a_ps", bufs=1, space="PSUM") as apool,
                tc.tile_pool(name="b_ps", bufs=1, space="PSUM") as bpool,
                tc.tile_pool(name="fin_sb", bufs=2) as fsb,
            ):
                def project_v(slab):
                    # v^T for 4 chunks (512 tokens) -> vta (fp8), strided
                    ps_v = spool.tile([C, 1024], F32, tag="sc", name="psv")
                    for i in range(4):
                        c = slab * 4 + i
                        nc.tensor.matmul(ps_v[:, ds(i * 128, HD)],
                                         lhsT=xs_sb[:, ts(c, 128)],
                                         rhs=w_sb[:, WV],
                                         start=True, stop=True)
                    src = ps_v[:, 0:512].rearrange(
                        "p (i x) -> p i x", x=128)[:, :, 0:HD]
                    dst1 = vta1_sb[:, ds(slab * 4 * 33, 4 * 33)].rearrange(
                        "p (c m) -> p c m", m=33)[:, :, 0:HD]
                    nc.scalar.copy(dst1, src)
                    dst8 = vta8_sb[:, ds(slab * 512, 512)].rearrange(
                        "p (c m) -> p c m", m=128)[:, :, 0:HD]
                    nc.scalar.copy(dst8, src)

                def project_q(j, on_act):
                    # t1|t2 for j-block j -> qq (bf16)
                    ps_q = spool.tile([C, 1024], F32, tag="sc", name="psq")
                    qoff = j * 512
                    nc.tensor.matmul(ps_q[:, 0:512], lhsT=w_sb[:, W1],
                                     rhs=xq_sb[:, ds(qoff, 512)],
                                     start=True, stop=True)
                    nc.tensor.matmul(ps_q[:, 512:1024], lhsT=w_sb[:, W2],
                                     rhs=xq_sb[:, ds(qoff, 512)],
                                     start=True, stop=True)
                    dst = qq_sb[:, ds(j * 1024, 1024)]
                    if on_act:
                        nc.scalar.copy(dst, ps_q[:, :])
                    else:
                        nc.vector.tensor_copy(dst, ps_q[:, :])

                def finalize_nb(nb):
                    # avs_sb [33|33, nb*1024 : +1024] -> out rows
                    CQ = NBS // 128  # 8 query chunks of 128
                    psT = spool.tile([C, 1024], F32, tag="sc", name="psT")
                    psT1 = psT[:, 0:512]
                    psT2 = psT[:, 512:1024]
                    for cq in range(CQ):
                        gq = nb * CQ + cq
                        nc.tensor.transpose(psT1[:, ds(cq * 64, 33)],
                                            avs_sb[0:33, ts(gq, 128)],
                                            id_sb[0:33, :])
                        nc.tensor.transpose(psT2[:, ds(cq * 64, 33)],
                                            avs_sb[64:97, ts(gq, 128)],
                                            id_sb[64:97, :])
                    r1_sb = fsb.tile([C, CQ], F32, tag="r1")
                    r2_sb = fsb.tile([C, CQ], F32, tag="r2")
                    sum1 = psT1.rearrange(
                        "p (c x) -> p c x", x=64)[:, :, 32:33]
                    sum2 = psT2.rearrange(
                        "p (c x) -> p c x", x=64)[:, :, 32:33]
                    nc.vector.reciprocal(r1_sb[:, :, None], sum1)
                    nc.vector.reciprocal(r2_sb[:, :, None], sum2)
                    nc.vector.tensor_scalar_mul(r2_sb[:, :], r2_sb[:, :],
                                                -LAMBDA)
                    o1_sb = fsb.tile([C, CQ * HD], F32, tag="o1")
                    o2_sb = fsb.tile([C, CQ * HD], F32, tag="o2")
                    av1t = psT1.rearrange(
                        "p (c x) -> p c x", x=64)[:, :, 0:32]
                    av2t = psT2.rearrange(
                        "p (c x) -> p c x", x=64)[:, :, 0:32]
                    o1_v = o1_sb[:, :].rearrange("p (c d) -> p c d", d=HD)
                    o2_v = o2_sb[:, :].rearrange("p (c d) -> p c d", d=HD)
                    nc.vector.tensor_tensor(
                        o1_v, av1t,
                        r1_sb[:, :, None].to_broadcast((C, CQ, HD)),
                        mybir.AluOpType.mult)
                    nc.vector.tensor_tensor(
                        o2_v, av2t,
                        r2_sb[:, :, None].to_broadcast((C, CQ, HD)),
                        mybir.AluOpType.mult)
                    nc.vector.tensor_tensor(
                        out_sb[:, ds(nb * CQ * HD, CQ * HD)],
                        o1_sb[:, :], o2_sb[:, :], mybir.AluOpType.add)
                    out_view = out_d[:, :].rearrange("(c p) d -> p c d", p=C)
                    nc.sync.dma_start(
                        out=out_view[:, nb * CQ:(nb + 1) * CQ, :],
                        in_=out_sb[:, ds(nb * CQ * HD, CQ * HD)]
                            .rearrange("p (c d) -> p c d", d=HD),
                    )

                # ---- preamble: v projections + first q projection
                for slab in range(8):
                    project_v(slab)
                project_q(0, True)

                # ---- main loop
                acc = 0.0
                pending_av = None    # (j, dc) whose avs are not yet emitted
                pending_drain = None  # j whose avA/avB need draining
                pending_fin = None   # nb awaiting finalize
                av_a = av_b = None

                for j in range(NJ):
                    for dc in range(ND):
                        c0, c1 = 2 * dc, 2 * dc + 1
                        # scores: chunk c0 then c1, shared lhsT per chunk
                        tiles = []
                        for c in (c0, c1):
                            T = spool.tile([C, 1024], F32, tag="sc")
                            nc.tensor.matmul(T[:, 0:512],
                                             lhsT=xs_sb[:, ts(c, 128)],
                                             rhs=qq_v[:, j, 0, :],
                                             start=True, stop=True)
                            nc.tensor.matmul(T[:, 512:1024],
                                             lhsT=xs_sb[:, ts(c, 128)],
                                             rhs=qq_v[:, j, 1, :],
                                             start=True, stop=True)
                            tiles.append((c, T))

                        # exp: whole-chunk, engine by weighted round-robin
                        for c, T in tiles:
                            s1_e = e1_sb[:, ds((c % ESLOTS) * 512, 512)]
                            s2_e = e2_sb[:, ds((c % ESLOTS) * 512, 512)]
                            acc += ACT_SHARE
                            if acc >= 1.0:
                                acc -= 1.0
                                nc.scalar.activation(s1_e, T[:, 0:512], Exp,
                                                     bias=bias_sb[:, 0:1],
                                                     scale=ACT_SCALE)
                                nc.scalar.activation(s2_e, T[:, 512:1024],
                                                     Exp,
                                                     bias=bias_sb[:, 0:1],
                                                     scale=ACT_SCALE)
                            else:
                                nc.vector.tensor_scalar(
                                    s1_e.bitcast(I16), T[:, 0:512],
                                    SCH16_A, SCH16_B,
                                    mybir.AluOpType.mult,
                                    mybir.AluOpType.add)
                                nc.vector.tensor_scalar(
                                    s2_e.bitcast(U8), T[:, 512:1024],
                                    SCH8_B, CLAMP8,
                                    mybir.AluOpType.add,
                                    mybir.AluOpType.min)

                        # av of the previous double-chunk
                        if pending_av is not None:
                            pj, pdc = pending_av
                            if pdc == 0:
                                av_a = apool.tile([C, 512], F32, tag="a")
                                av_b = bpool.tile([C, 512], F32, tag="b")
                            first, last = (pdc == 0), (pdc == ND - 1)
                            for pc in (2 * pdc, 2 * pdc + 1):
                                nc.tensor.matmul(
                                    av_a[0:33, :],
                                    lhsT=vta1_sb[:, ds(pc * 33, 33)],
                                    rhs=e1_sb[:, ds((pc % ESLOTS) * 512,
                                                    512)],
                                    start=(first and pc % 2 == 0),
                                    stop=(last and pc % 2 == 1),
                                    skip_group_check=True)
                            sl0 = (2 * pdc) % ESLOTS
                            rhs2 = e2_sb[:, ds(sl0 * 512, 1024)] \
                                .rearrange("p (s x) -> p s x", s=2)
                            nc.tensor.matmul(
                                av_b[:, :],
                                lhsT=vta8_v[:, pdc, :, :],
                                rhs=rhs2,
                                start=first, stop=last,
                                perf_mode=DR,
                                skip_group_check=True)
                            pending_av = None

                        pending_av = (j, dc)

                        # staggered boundary work
                        if dc == 0 and pending_drain is not None:
                            pj = pending_drain
                            nc.scalar.copy(avs_sb[0:33, ts(pj, 512)],
                                           av_a[0:33, :])
                            nc.vector.tensor_copy(avs_sb[64:97, ts(pj, 512)],
                                                  av_b[0:33, :])
                            pending_drain = None
                            if pj % 2 == 1:
                                pending_fin = pj // 2
                        if dc == 1 and pending_fin is not None:
                            finalize_nb(pending_fin)
                            pending_fin = None
                        if dc == 12 and j + 1 < NJ:
                            project_q(j + 1, acc < 0.5)

                    # j done: next iteration's dc==0 emits the last avs;
                    # note the drain for after them
                    pending_drain = j

                # flush: last dc's avs, drain, finalize
                pj, pdc = pending_av
                for pc in (2 * pdc, 2 * pdc + 1):
                    nc.tensor.matmul(
                        av_a[0:33, :],
                        lhsT=vta1_sb[:, ds(pc * 33, 33)],
                        rhs=e1_sb[:, ds((pc % ESLOTS) * 512, 512)],
                        start=False, stop=(pc % 2 == 1),
                        skip_group_check=True)
                sl0 = (2 * pdc) % ESLOTS
                rhs2 = e2_sb[:, ds(sl0 * 512, 1024)] \
                    .rearrange("p (s x) -> p s x", s=2)
                nc.tensor.matmul(av_b[:, :], lhsT=vta8_v[:, pdc, :, :],
                                 rhs=rhs2, start=False, stop=True,
                                 perf_mode=DR, skip_group_check=True)
                nc.scalar.copy(avs_sb[0:33, ts(pj, 512)], av_a[0:33, :])
                nc.vector.tensor_copy(avs_sb[64:97, ts(pj, 512)],
                                      av_b[0:33, :])
                finalize_nb(pj // 2)

    nc.compile()
    return nc


def make_identity_input():
    ident = np.zeros((C, 33), np.float32)
    ident[0:33, :] = np.eye(33, dtype=np.float32)
    ident[64:97, :] = np.eye(33, dtype=np.float32)
    return ident


def make_in_maps(x, w_qkv):
    """Host-side sharding: subsample, fold score weights, pack per core."""
    xs = np.ascontiguousarray(x[0][:, ::R, ::R, ::R]).reshape(C, N)
    xs_b = xs.astype(NP_BF16)
    ident = make_identity_input()
    in_maps = []
    for core in range(N_CORES):
        h, half = divmod(core, 2)
        blk = w_qkv[h * 96: (h + 1) * 96].astype(np.float64)
        wq, wk, wv = blk[0:32], blk[32:64], blk[64:96]
        # folded, pre-scaled score weights: t_h = (c8 wq_h^T wk_h)^T? see
        # build_nc: lhsT for the t-projection must be W_h^T = wq_h^T wk_h
        w1t = C8 * (wq[0:DH].T @ wk[0:DH])        # [128, 128]
        w2t = C8 * (wq[DH:HD].T @ wk[DH:HD])
        w = np.empty((C, 288), np.float32)
        w[:, 0:128] = w1t
        w[:, 128:256] = w2t
        w[:, 256:288] = wv.T
        in_maps.append({
            "xs": xs_b,
            "xq": np.ascontiguousarray(xs_b[:, half * NQ:(half + 1) * NQ]),
            "w": w.astype(NP_BF16),
            "ident": ident,
        })
    return in_maps


_NC_CACHE = {}


def get_nc():
    if "nc" not in _NC_CACHE:
        _NC_CACHE["nc"] = build_nc()
    return _NC_CACHE["nc"]


LAST_RESULTS = None  # BassKernelResults of the most recent kernel() call


def kernel(x, w_qkv, trace=False, **trace_kwargs):
    global LAST_RESULTS
    x = np.asarray(x)
    w_qkv = np.asarray(w_qkv)
    in_maps = make_in_maps(x, w_qkv)
    nc = get_nc()
    res = run_bass_kernel_spmd(nc, in_maps, list(range(N_CORES)),
                               trace=trace, **trace_kwargs)
    LAST_RESULTS = res
    out_hnd = np.empty((HEADS, N, HD), np.float32)
    for core in range(N_CORES):
        h, half = divmod(core, 2)
        out_hnd[h, half * NQ:(half + 1) * NQ, :] = res.results[core]["out"]
    return out_hnd.reshape(1, C, 16, 16, 16)
